# revision 17
# baseline (speedup 1.0000x reference)
"""2-layer GCN (GCNConv + LayerNorm + ReLU + GCNConv + LayerNorm) on 8 TRN2 NeuronCores.

Strategy (v2):
  - Nodes are degree-sorted and dealt round-robin to 8 cores (uniform degree profiles
    -> identical SPMD schedules). Each core owns 6250 dst nodes (padded to 6272).
  - Per layer: scale local rows by dinv, cast bf16, AllGather -> full 50176-row table
    in each core's DRAM.
  - Aggregation: edges grouped by (dst tile, round) with lane == dst position, so a
    gathered 128-edge chunk accumulates into PSUM via a matmul with a *constant
    identity* stationary: psum[d, f] += G[d, f]. Source rows fetched by gpsimd
    dma_gather (int16 indices -> table split into two 25088-row halves; dst nodes
    re-tiled per half by half-degree to keep padding low). The half-1 partial sums
    are folded into half-0 (storage) tile order with HOST-built permutation matrices
    (DMA'd, not built on DVE).
  - Gathers use prepare_only + trigger_dma so Q7 descriptor generation (the
    bottleneck, ~7ns/row) runs continuously, overlapped with collectives/compute.
  - Self-loops are added locally (identity matmul of the resident scaled rows).
  - Dense W matmul per tile (transpose via PE), then LayerNorm on f32.
"""
import os
import numpy as np
import ml_dtypes

N = 50000
E = 600000
D = 128
NC = 8
P = 128
SHARD = 6272            # 49 * 128
TILES = 49
HALF_ROWS = SHARD * 4   # 25088 rows per gather half (< int16 range)
LN_EPS = 1e-5
GBUF_CHUNKS = 64        # chunks (128 edges each) per dma_gather call group
PREP_AHEAD = 3          # gather groups desc-prepped ahead of their trigger

bf16 = ml_dtypes.bfloat16


# ----------------------------------------------------------------------------
# Host-side planning (index-only preprocessing)
# ----------------------------------------------------------------------------

class Plan:
    pass


def build_plan(edge_index: np.ndarray) -> Plan:
    pl = Plan()
    src = edge_index[0].astype(np.int64)
    dst = edge_index[1].astype(np.int64)

    deg = np.bincount(dst, minlength=N) + 1          # incl. mandatory self-loop
    order = np.argsort(-deg, kind="stable")          # global degree desc
    core_of = np.empty(N, dtype=np.int64)
    core_of[order] = np.arange(N) % NC               # deal round-robin

    src_half = (core_of[src] >= 4).astype(np.int64)  # 0: table rows 0..25087
    degH = np.zeros((2, N), dtype=np.int64)
    degH[0] = np.bincount(dst[src_half == 0], minlength=N)
    degH[1] = np.bincount(dst[src_half == 1], minlength=N)

    # Storage order per core = H0 order (sorted by degH0 desc); H1 order separate.
    posH = np.empty((2, N), dtype=np.int64)
    node_at = np.full((NC, SHARD), -1, dtype=np.int64)   # storage order
    for c in range(NC):
        shard = order[c::NC]                              # 6250 nodes
        for h in range(2):
            so = np.argsort(-degH[h][shard], kind="stable")
            posH[h][shard[so]] = np.arange(len(shard))
        node_at[c, :len(shard)] = shard[np.argsort(posH[0][shard])]
    pl.node_at = node_at
    pl.deg = deg

    lane_of = posH % P
    tile_of = posH // P

    # per (half, tile): R = max lane count, uniform over cores
    R_uni = np.zeros((2, TILES), dtype=np.int64)
    for h in range(2):
        key = core_of * TILES + tile_of[h]
        m = np.zeros(NC * TILES, dtype=np.int64)
        np.maximum.at(m, key, degH[h])
        R_uni[h] = m.reshape(NC, TILES).max(axis=0)
    pl.R_uni = R_uni

    chunk_base = np.zeros((2, TILES + 1), dtype=np.int64)
    for h in range(2):
        chunk_base[h, 1:] = np.cumsum(R_uni[h])
    pl.chunk_base = chunk_base
    pl.n_chunks = chunk_base[:, -1]

    # round index for each edge: rank among edges with same (half, dst)
    ekey = src_half * N + dst
    eorder = np.argsort(ekey, kind="stable")
    sk = ekey[eorder]
    starts = np.r_[0, np.flatnonzero(sk[1:] != sk[:-1]) + 1]
    group_of = np.zeros(E, dtype=np.int64)
    group_of[starts[1:]] = 1
    group_of = np.cumsum(group_of)
    rounds_sorted = np.arange(E) - starts[group_of]
    rounds = np.empty(E, dtype=np.int64)
    rounds[eorder] = rounds_sorted

    # slot arrays per (core, half): [n_chunks*128] of table indices (pad -> zero row)
    PAD_IDX = 6250                                    # dummy (zero) row
    slots = [[np.full(pl.n_chunks[h] * P, PAD_IDX, dtype=np.int64) for h in range(2)]
             for _ in range(NC)]
    e_core = core_of[dst]
    e_tile = tile_of[src_half, dst]
    e_lane = lane_of[src_half, dst]
    e_slot = (chunk_base[src_half, e_tile] + rounds) * P + e_lane
    e_val = (core_of[src] % 4) * SHARD + posH[0][src]
    for c in range(NC):
        for h in range(2):
            m = (e_core == c) & (src_half == h)
            slots[c][h][e_slot[m]] = e_val[m]

    # call groups per half: greedy fill up to GBUF_CHUNKS
    groups = [[], []]
    for h in range(2):
        cur, cur_n = [], 0
        for t in range(TILES):
            r = int(R_uni[h][t])
            if r == 0:
                continue
            while r > GBUF_CHUNKS:
                if cur:
                    groups[h].append(cur)
                    cur, cur_n = [], 0
                groups[h].append([(t, GBUF_CHUNKS, True)])
                r -= GBUF_CHUNKS
            if cur_n + r > GBUF_CHUNKS and cur:
                groups[h].append(cur)
                cur, cur_n = [], 0
            cur.append((t, r, False))
            cur_n += r
        if cur:
            groups[h].append(cur)
    pl.groups = groups
    # schedule: all half-1 groups first (builds H1sb), then half-0 descending:
    # the tail then holds only the last group's few (heavy) tiles.
    sched = [(1, gi) for gi in range(len(groups[1]))]
    sched += [(0, gi) for gi in reversed(range(len(groups[0])))]
    pl.sched = sched

    def wrap(flat):                                    # [num] -> [128, num//16]
        num = len(flat)
        w = np.zeros((16, num // 16), dtype=np.int16)
        w[np.arange(num) % 16, np.arange(num) // 16] = flat.astype(np.int16)
        return np.tile(w, (8, 1))

    # wrapped int16 index tensors [128, total_cols] per core; groups consume
    # consecutive chunk ranges per half, tracked via per-half cursors.
    idx_in = []
    col_ranges = [[], []]
    for c in range(NC):
        parts = []
        col = 0
        cursor = [0, 0]                                # chunk cursor per half
        for h in range(2):
            for grp in groups[h]:
                nch = sum(r for (_, r, _) in grp)
                i0 = cursor[h] * P
                i1 = (cursor[h] + nch) * P
                cursor[h] += nch
                seg = slots[c][h][i0:i1]
                parts.append(wrap(seg))
                if c == 0:
                    col_ranges[h].append((col, col + len(seg) // 16))
                col += len(seg) // 16
        idx_in.append(np.concatenate(parts, axis=1))
    pl.idx_in = idx_in
    pl.col_ranges = col_ranges

    # permutation blocks: fold H1-tiling partials into H0 (storage) tiling.
    # Uniform schedule: per tile t, the list of source H1 tiles = union over cores.
    need = [set() for _ in range(TILES)]
    percore = [[dict() for _ in range(TILES)] for _ in range(NC)]
    for c in range(NC):
        for pidx in range(SHARD):
            n_ = node_at[c, pidx]
            if n_ < 0:
                continue
            q = posH[1][n_]
            t, l = pidx // P, pidx % P
            t1, l1 = q // P, q % P
            need[t].add(t1)
            percore[c][t].setdefault(t1, []).append((l1, l))
    block_t1 = [sorted(need[t]) for t in range(TILES)]
    NBLK = np.array([len(s) for s in block_t1], dtype=np.int64)
    TB = int(NBLK.sum())
    # perm shipped partition-major: perm2[l1, bi*P + l] so the per-tile DMA load
    # reads contiguous bytes per partition.
    perm_in = []
    for c in range(NC):
        pm = np.zeros((P, TB * P), dtype=np.float32)
        bi = 0
        for t in range(TILES):
            for t1 in block_t1[t]:
                for (l1, l) in percore[c][t].get(t1, []):
                    pm[l1, bi * P + l] = 1.0
                bi += 1
        perm_in.append(pm.astype(bf16))
    pl.NBLK = NBLK
    pl.TB = TB
    pl.perm_in = perm_in
    pl.block_t1 = block_t1
    return pl


# ----------------------------------------------------------------------------
# Bass kernel
# ----------------------------------------------------------------------------

def build_bass(pl, triv):
    import concourse.bacc as bacc
    import concourse.mybir as mybir
    import concourse.tile as tile
    from concourse.masks import make_identity

    f32 = mybir.dt.float32
    bf = mybir.dt.bfloat16
    AF = mybir.ActivationFunctionType
    OP = mybir.AluOpType

    nc = bacc.Bacc("TRN2", target_bir_lowering=False, debug=False, num_devices=NC)

    x_ext = nc.declare_dram_parameter("x", [SHARD, D], f32, isOutput=False)
    deg_ext = nc.declare_dram_parameter("deg", [P, TILES], f32, isOutput=False)
    totcols = pl.idx_in[0].shape[1]
    idx_ext = nc.declare_dram_parameter("idx", [P, totcols], mybir.dt.int16, isOutput=False)
    perm_ext = nc.declare_dram_parameter("perm", [P, pl.TB * P], bf, isOutput=False)
    W_ext = [nc.declare_dram_parameter(f"W{i+1}", [D, D], f32, isOutput=False) for i in range(2)]
    vecs_ext = {}
    for nm in ["b1", "g1", "beta1", "b2", "g2", "beta2"]:
        vecs_ext[nm] = nc.declare_dram_parameter(nm, [P, D], f32, isOutput=False)
    out_ext = nc.declare_dram_parameter("out", [SHARD, D], f32, isOutput=True)

    NBMAX = int(pl.NBLK.max())

    with tile.TileContext(nc) as tc:
        with tc.tile_pool(name="const", bufs=1) as cpool, \
             tc.tile_pool(name="store", bufs=1) as spool, \
             tc.tile_pool(name="g", bufs=4) as gpool, \
             tc.tile_pool(name="work", bufs=3) as wpool, \
             tc.tile_pool(name="permp", bufs=4) as ppool, \
             tc.tile_pool(name="psA", bufs=4, space="PSUM") as psA, \
             tc.tile_pool(name="psB", bufs=2, space="PSUM") as psB, \
             tc.tile_pool(name="psC", bufs=2, space="PSUM") as psC, \
             tc.tile_pool(name="dram", bufs=1, space="DRAM") as dpool:

            ident32 = cpool.tile([P, P], f32)
            make_identity(nc, ident32[:])
            ident_bf = cpool.tile([P, P], bf)
            nc.vector.tensor_copy(out=ident_bf[:], in_=ident32[:])

            Wbf = []
            for i in range(2):
                wt = cpool.tile([P, D], f32, name=f"w32_{i}")
                nc.sync.dma_start(out=wt[:], in_=W_ext[i][:])
                wb = cpool.tile([P, D], bf, name=f"wbf_{i}")
                nc.vector.tensor_copy(out=wb[:], in_=wt[:])
                Wbf.append(wb)

            vecs = {}
            for nm in vecs_ext:
                vt = cpool.tile([P, D], f32, name=f"vec_{nm}")
                nc.sync.dma_start(out=vt[:], in_=vecs_ext[nm][:])
                vecs[nm] = vt

            deg_t = cpool.tile([P, TILES], f32)
            nc.sync.dma_start(out=deg_t[:], in_=deg_ext[:])
            sq = cpool.tile([P, TILES], f32)
            nc.scalar.activation(out=sq[:], in_=deg_t[:], func=AF.Sqrt)
            dinv = cpool.tile([P, TILES], f32)
            nc.vector.reciprocal(dinv[:], sq[:])

            eps_t = cpool.tile([P, 1], f32)
            nc.vector.memset(eps_t[:], float(LN_EPS))
            # per-partition mask: 1.0 for real lanes of the last tile, 0.0 for dummies
            ndum = SHARD - 6250
            dmask = cpool.tile([P, 1], f32)
            nc.vector.memset(dmask[:], 1.0)
            nc.gpsimd.affine_select(
                out=dmask[:], in_=dmask[:], pattern=[[0, 1]],
                compare_op=OP.is_ge, fill=0.0,
                base=P - ndum - 1, channel_multiplier=-1)
            idx_t = cpool.tile([P, totcols], mybir.dt.int16)
            nc.sync.dma_start(out=idx_t[:], in_=idx_ext[:])

            x_store = spool.tile([P, TILES, D], f32)
            xs_store = spool.tile([P, TILES, D], bf)
            for t0 in range(0, TILES, 7):
                t1b = min(t0 + 7, TILES)
                nc.sync.dma_start(
                    out=x_store[:, t0:t1b, :],
                    in_=x_ext[t0 * P:t1b * P, :].rearrange(
                        "(t l) f -> l t f", t=t1b - t0))
                for t in range(t0, t1b):
                    nc.scalar.activation(out=xs_store[:, t, :], in_=x_store[:, t, :],
                                         func=AF.Identity, scale=dinv[:, t:t + 1])

            dinvm = cpool.tile([P, 1], f32)
            nc.vector.tensor_scalar(out=dinvm[:], in0=dinv[:, TILES - 1:TILES],
                                    scalar1=dmask[:, 0:1], scalar2=None, op0=OP.mult)
            h1g_store = spool.tile([P, TILES, D], bf)
            H1sb = spool.tile([P, TILES, D], bf)

            cc_in = [dpool.tile([SHARD, D], bf, name=f"ccin{i}") for i in range(2)]
            cc_out = [dpool.tile([NC * SHARD, D], bf, name=f"ccout{i}",
                                 addr_space="Shared") for i in range(2)]

            def finish_h0_tile(layer, t, ps, started):
                selfstore = xs_store if layer == 0 else h1g_store
                nc.tensor.matmul(out=ps[:], lhsT=ident_bf[:],
                                 rhs=selfstore[:, t, :], start=not started, stop=False)
                bi0 = int(pl.NBLK[:t].sum())
                nb = len(pl.block_t1[t])
                assert nb >= 1
                pb = ppool.tile([P, NBMAX, P], bf, tag="pb", name=f"pb_{layer}_{t}")
                nc.sync.dma_start(out=pb[:, :nb, :],
                                  in_=perm_ext[:, bi0 * P:(bi0 + nb) * P])
                for j, t1 in enumerate(pl.block_t1[t]):
                    nc.tensor.matmul(out=ps[:], lhsT=pb[:, j, :], rhs=H1sb[:, t1, :],
                                     start=False, stop=(j == nb - 1))

                # evict agg -> bf16 (no dinv scale: LN is scale-invariant when b==0)
                b_triv, g_triv, be_triv = triv[layer]
                s_agg = wpool.tile([P, D], bf, tag="sagg", name=f"sagg_{layer}_{t}")
                if b_triv:
                    nc.scalar.activation(out=s_agg[:], in_=ps[:], func=AF.Identity)
                else:
                    nc.scalar.activation(out=s_agg[:], in_=ps[:], func=AF.Identity,
                                         scale=dinv[:, t:t + 1])
                psT = psB.tile([P, D], bf, space="PSUM", tag="tr",
                               name=f"psT_{layer}_{t}")
                nc.tensor.transpose(out=psT[:], in_=s_agg[:], identity=ident_bf[:])
                s_aggT = wpool.tile([P, D], bf, tag="saggT", name=f"saggT_{layer}_{t}")
                nc.vector.tensor_copy(out=s_aggT[:], in_=psT[:])
                convp = psC.tile([P, D], f32, space="PSUM", tag="conv",
                                 name=f"conv_{layer}_{t}")
                nc.tensor.matmul(out=convp[:], lhsT=s_aggT[:], rhs=Wbf[layer][:],
                                 start=True, stop=True)

                if b_triv:
                    cb_ap = convp[:]
                else:
                    bv = vecs["b1" if layer == 0 else "b2"]
                    cb = wpool.tile([P, D], f32, tag="cb", name=f"cb_{layer}_{t}")
                    nc.vector.tensor_tensor(out=cb[:], in0=convp[:], in1=bv[:], op=OP.add)
                    cb_ap = cb[:]
                scr = wpool.tile([P, D], f32, tag="scr", name=f"scr_{layer}_{t}")
                negmu = wpool.tile([P, 1], f32, tag="negmu", name=f"negmu_{layer}_{t}")
                nc.scalar.activation(out=scr[:], in_=cb_ap, func=AF.Identity,
                                     scale=-1.0 / D, accum_out=negmu[:])
                ctr = wpool.tile([P, D], f32, tag="ctr", name=f"ctr_{layer}_{t}")
                nc.scalar.activation(out=ctr[:], in_=cb_ap, func=AF.Identity,
                                     bias=negmu[:, 0:1])
                sqs = wpool.tile([P, D], f32, tag="sqs", name=f"sqs_{layer}_{t}")
                var_raw = wpool.tile([P, 1], f32, tag="varr", name=f"varr_{layer}_{t}")
                nc.scalar.activation(out=sqs[:], in_=ctr[:], func=AF.Square,
                                     scale=float(1.0 / np.sqrt(D)),
                                     accum_out=var_raw[:])
                std = wpool.tile([P, 1], f32, tag="std", name=f"std_{layer}_{t}")
                nc.scalar.activation(out=std[:], in_=var_raw[:], func=AF.Sqrt,
                                     bias=eps_t[:, 0:1])
                rstd = wpool.tile([P, 1], f32, tag="rstd", name=f"rstd_{layer}_{t}")
                nc.vector.reciprocal(rstd[:], std[:])

                if not (g_triv and be_triv):
                    gv = vecs["g1" if layer == 0 else "g2"]
                    bev = vecs["beta1" if layer == 0 else "beta2"]
                    o1 = wpool.tile([P, D], f32, tag="o1", name=f"o1_{layer}_{t}")
                    nc.scalar.activation(out=o1[:], in_=ctr[:], func=AF.Identity,
                                         scale=rstd[:, 0:1])
                    o2 = wpool.tile([P, D], f32, tag="o2", name=f"o2_{layer}_{t}")
                    nc.vector.tensor_tensor(out=o2[:], in0=o1[:], in1=gv[:], op=OP.mult)
                    o3 = wpool.tile([P, D], f32, tag="o3", name=f"o3_{layer}_{t}")
                    nc.vector.tensor_tensor(out=o3[:], in0=o2[:], in1=bev[:], op=OP.add)
                    if layer == 0:
                        o4 = wpool.tile([P, D], f32, tag="o4", name=f"o4_{t}")
                        nc.scalar.activation(out=o4[:], in_=o3[:], func=AF.Relu)
                        dcol = dinvm[:, 0:1] if t == TILES - 1 else dinv[:, t:t + 1]
                        nc.vector.tensor_scalar(out=h1g_store[:, t, :], in0=o4[:],
                                                scalar1=dcol, scalar2=None,
                                                op0=OP.mult)
                    else:
                        nc.sync.dma_start(out=out_ext[t * P:(t + 1) * P, :], in_=o3[:])
                else:
                    if layer == 0:
                        dcol = dinvm[:, 0:1] if t == TILES - 1 else dinv[:, t:t + 1]
                        rsd = wpool.tile([P, 1], f32, tag="rsd", name=f"rsd_{t}")
                        nc.vector.tensor_scalar(out=rsd[:], in0=rstd[:],
                                                scalar1=dcol, scalar2=None,
                                                op0=OP.mult)
                        nc.scalar.activation(out=h1g_store[:, t, :], in_=ctr[:],
                                             func=AF.Relu, scale=rsd[:, 0:1])
                    else:
                        o1 = wpool.tile([P, D], f32, tag="o1", name=f"o1_{layer}_{t}")
                        nc.scalar.activation(out=o1[:], in_=ctr[:], func=AF.Identity,
                                             scale=rstd[:, 0:1])
                        nc.sync.dma_start(out=out_ext[t * P:(t + 1) * P, :], in_=o1[:])

            def run_layer(layer):
                selfstore = xs_store if layer == 0 else h1g_store
                table = cc_out[layer]

                nsched = len(pl.sched)

                for t0 in range(0, TILES, 7):
                    t1b = min(t0 + 7, TILES)
                    nc.sync.dma_start(
                        out=cc_in[layer][t0 * P:t1b * P, :].rearrange(
                            "(t l) f -> l t f", t=t1b - t0),
                        in_=selfstore[:, t0:t1b, :])
                nc.gpsimd.collective_compute(
                    "AllGather", OP.bypass,
                    replica_groups=[list(range(NC))],
                    ins=[cc_in[layer][:].opt()],
                    outs=[cc_out[layer][:].opt()],
                )
                nc.vector.memset(H1sb[:], 0.0)

                open_ps = {}                 # (h, t) -> (ps tile, started)
                for k in range(nsched):
                    h, gi = pl.sched[k]
                    grp = pl.groups[h][gi]
                    half_ap = table[HALF_ROWS:, :] if h == 1 else table[:HALF_ROWS, :]
                    c0, c1 = pl.col_ranges[h][gi]
                    nch = sum(r for (_, r, _) in grp)
                    gbuf = gpool.tile([P, GBUF_CHUNKS, D], bf, tag="g",
                                      name=f"g_{layer}_{h}_{gi}")
                    nc.gpsimd.dma_gather(
                        out_ap=gbuf[:, :nch, :],
                        in_ap=half_ap,
                        idxs_ap=idx_t[:, c0:c1],
                        num_idxs=nch * P,
                        num_idxs_reg=nch * P,
                        elem_size=D,
                        single_packet=False,
                    )
                    off = 0
                    for (t, r, partial) in grp:
                        key = (h, t)
                        if key in open_ps:
                            ps, started = open_ps.pop(key)
                        else:
                            ps = psA.tile([P, D], f32, space="PSUM", tag="agg",
                                          name=f"ps_{layer}_{h}_{t}")
                            started = False
                        for ri in range(r):
                            last = (not partial) and (h == 1) and (ri == r - 1)
                            nc.tensor.matmul(out=ps[:], lhsT=ident_bf[:],
                                             rhs=gbuf[:, off + ri, :],
                                             start=not started, stop=last)
                            started = True
                        off += r
                        if partial:
                            open_ps[key] = (ps, started)
                        elif h == 1:
                            nc.scalar.activation(out=H1sb[:, t, :], in_=ps[:],
                                                 func=AF.Identity)
                        else:
                            finish_h0_tile(layer, t, ps, started)

                covered = set(t for grp in pl.groups[0] for (t, _, pa) in grp if not pa)
                for t in range(TILES):
                    if t not in covered:
                        ps = psA.tile([P, D], f32, space="PSUM", tag="agg",
                                      name=f"ps_{layer}_0z_{t}")
                        finish_h0_tile(layer, t, ps, False)

            run_layer(0)
            run_layer(1)

    nc.compile()
    return nc


# ----------------------------------------------------------------------------
# Entry point
# ----------------------------------------------------------------------------

_last_result = None


def kernel(**inputs) -> np.ndarray:
    edge_index = np.asarray(inputs["edge_index"])
    pl = build_plan(edge_index)

    from concourse.bass_utils import run_bass_kernel_spmd
    triv = []
    for i in (1, 2):
        triv.append((
            not np.any(np.asarray(inputs[f"b{i}"])),
            np.all(np.asarray(inputs[f"g{i}"]) == 1.0),
            not np.any(np.asarray(inputs[f"beta{i}"])),
        ))
    nc = build_bass(pl, triv)

    x = np.asarray(inputs["x"], dtype=np.float32)
    in_maps = []
    for c in range(NC):
        deg_t = np.ones((P, TILES), dtype=np.float32)
        xp = np.zeros((SHARD, D), dtype=np.float32)
        valid = pl.node_at[c] >= 0
        pidx = np.arange(SHARD)
        deg_t[pidx[valid] % P, pidx[valid] // P] = pl.deg[pl.node_at[c][valid]]
        xp[valid] = x[pl.node_at[c][valid]]
        m = {
            "x": xp,
            "deg": deg_t,
            "idx": pl.idx_in[c],
            "perm": pl.perm_in[c],
            "W1": np.asarray(inputs["W1"], np.float32),
            "W2": np.asarray(inputs["W2"], np.float32),
        }
        for nm in ["b1", "g1", "beta1", "b2", "g2", "beta2"]:
            m[nm] = np.tile(np.asarray(inputs[nm], np.float32)[None, :], (P, 1))
        in_maps.append(m)

    kw = {}
    if os.environ.get("KERNEL_TRACE") == "1":
        kw = dict(trace=True, trace_cores=[0])
    res = run_bass_kernel_spmd(nc, in_maps, core_ids=list(range(NC)), **kw)
    global _last_result
    _last_result = res

    out = np.zeros((N, D), dtype=np.float32)
    for c in range(NC):
        o = np.asarray(res.results[c]["out"], dtype=np.float32)
        valid = pl.node_at[c] >= 0
        out[pl.node_at[c][valid]] = o[valid]
    return out


# revision 19
# speedup vs baseline: 1.0980x; 1.0980x over previous
"""2-layer GCN (GCNConv + LayerNorm + ReLU + GCNConv + LayerNorm) on 8 TRN2 NeuronCores.

Strategy (v3, split tables):
  - Nodes degree-sorted, dealt round-robin to 8 cores (uniform SPMD schedules).
  - Gather tables split by STORAGE POSITION range (not core group):
      table-P = all cores' positions [0,4096)  -> 8*4096 = 32768 rows (int16 max)
      table-S = all cores' positions [4096,6272) -> 8*2176 = 17408 rows
    P-members = each core's top-4095 nodes by total degree (pos 4095 = zero dummy);
    storage order sorted by degP within each segment so the P-gather needs NO fold.
    S-gather uses its own degS-sorted dst tiling, folded into storage tiling with
    host-built permutation matrices (DMA'd).
  - Each layer's AllGather splits into AG-P (input tiles 0-31) and AG-S (tiles
    32-48). Layer-2's AG-S fires mid-layer-1 (its input tiles finish first in the
    P phase), AG-P at layer-1's end -> layer-2 desc-gen starts almost immediately.
  - Per-layer schedule: all S-groups (build Ssb), then P-groups with tiles 32-48
    first (their finishes feed the next layer's AG-S), then tiles 0-31.
  - Aggregation: lane == dst position chunks accumulate into PSUM via matmuls with
    a constant identity stationary; gpsimd dma_gather fetches rows (~7.3ns/idx Q7).
  - Self-loops added locally; dense W matmul per tile (transpose via PE); LN f32.
"""
import os
import numpy as np
import ml_dtypes

N = 50000
E = 600000
D = 128
NC = 8
P = 128
SHARD = 6272            # 49 * 128
TILES = 49
PREF = 4096             # storage positions in table-P (32 tiles)
SUF = SHARD - PREF      # 2176 positions in table-S (17 tiles)
NP_MEM = 4095           # real nodes in P per core (pos 4095 = zero dummy)
PTILES = PREF // P      # 32
LN_EPS = 1e-5
GBUF_CHUNKS = 64        # chunks (128 edges each) per dma_gather call group

bf16 = ml_dtypes.bfloat16


# ----------------------------------------------------------------------------
# Host-side planning (index-only preprocessing)
# ----------------------------------------------------------------------------

class Plan:
    pass


def build_plan(edge_index: np.ndarray) -> Plan:
    pl = Plan()
    src = edge_index[0].astype(np.int64)
    dst = edge_index[1].astype(np.int64)

    deg = np.bincount(dst, minlength=N) + 1          # incl. mandatory self-loop
    order = np.argsort(-deg, kind="stable")          # global degree desc
    core_of = np.empty(N, dtype=np.int64)
    core_of[order] = np.arange(N) % NC               # deal round-robin
    pc_rank = np.empty(N, dtype=np.int64)            # per-core degree rank
    pc_rank[order] = np.arange(N) // NC

    in_P = pc_rank < NP_MEM                          # table membership (fixed)
    e_half = np.where(in_P[src], 0, 1)               # 0: P, 1: S
    degH = np.zeros((2, N), dtype=np.int64)
    degH[0] = np.bincount(dst[e_half == 0], minlength=N)
    degH[1] = np.bincount(dst[e_half == 1], minlength=N)

    # storage position: P-members sorted by degP desc at [0,4095), dummy at 4095,
    # S-members sorted by degP desc at [4096,6250+...). S dst tiling by degS.
    posStore = np.full(N, -1, dtype=np.int64)
    posS = np.full(N, -1, dtype=np.int64)
    node_at = np.full((NC, SHARD), -1, dtype=np.int64)
    for c in range(NC):
        shard = order[c::NC]                          # 6250, degree desc
        pm = shard[pc_rank[shard] < NP_MEM]           # 4095 P-members
        sm = shard[pc_rank[shard] >= NP_MEM]          # 2155 S-members
        pm = pm[np.argsort(-degH[0][pm], kind="stable")]
        sm = sm[np.argsort(-degH[0][sm], kind="stable")]
        posStore[pm] = np.arange(len(pm))
        posStore[sm] = PREF + np.arange(len(sm))
        node_at[c, :len(pm)] = pm
        node_at[c, PREF:PREF + len(sm)] = sm
        so = np.argsort(-degH[1][shard], kind="stable")
        posS[shard[so]] = np.arange(len(shard))
    pl.node_at = node_at
    pl.deg = deg

    # half-0 (P) dst tiling = storage tiling; half-1 (S) dst tiling = posS
    tile_of = np.stack([posStore // P, posS // P])
    lane_of = np.stack([posStore % P, posS % P])

    # per (half, tile): R = max lane count, uniform over cores
    R_uni = np.zeros((2, TILES), dtype=np.int64)
    for h in range(2):
        key = core_of * TILES + tile_of[h]
        m = np.zeros(NC * TILES, dtype=np.int64)
        np.maximum.at(m, key, degH[h])
        R_uni[h] = m.reshape(NC, TILES).max(axis=0)
    pl.R_uni = R_uni

    chunk_base = np.zeros((2, TILES + 1), dtype=np.int64)
    for h in range(2):
        chunk_base[h, 1:] = np.cumsum(R_uni[h])
    pl.chunk_base = chunk_base
    pl.n_chunks = chunk_base[:, -1]

    # round index for each edge: rank among edges with same (half, dst)
    ekey = e_half * N + dst
    eorder = np.argsort(ekey, kind="stable")
    sk = ekey[eorder]
    starts = np.r_[0, np.flatnonzero(sk[1:] != sk[:-1]) + 1]
    group_of = np.zeros(E, dtype=np.int64)
    group_of[starts[1:]] = 1
    group_of = np.cumsum(group_of)
    rounds_sorted = np.arange(E) - starts[group_of]
    rounds = np.empty(E, dtype=np.int64)
    rounds[eorder] = rounds_sorted

    # slot arrays per (core, half): table indices (pad -> zero row)
    PAD_P = NP_MEM                                    # core 0 pos 4095 (zero)
    PAD_S = 6251 - PREF                               # core 0 pos 6251 (zero)
    slots = [[np.full(pl.n_chunks[0] * P, PAD_P, dtype=np.int64),
              np.full(pl.n_chunks[1] * P, PAD_S, dtype=np.int64)]
             for _ in range(NC)]
    e_core = core_of[dst]
    e_tile = tile_of[e_half, dst]
    e_lane = lane_of[e_half, dst]
    e_slot = (chunk_base[e_half, e_tile] + rounds) * P + e_lane
    e_val = np.where(e_half == 0,
                     core_of[src] * PREF + posStore[src],
                     core_of[src] * SUF + (posStore[src] - PREF))
    for c in range(NC):
        for h in range(2):
            m = (e_core == c) & (e_half == h)
            slots[c][h][e_slot[m]] = e_val[m]

    # call groups per half: greedy fill up to GBUF_CHUNKS
    groups = [[], []]
    for h in range(2):
        cur, cur_n = [], 0
        for t in range(TILES):
            r = int(R_uni[h][t])
            if r == 0:
                continue
            while r > GBUF_CHUNKS:
                if cur:
                    groups[h].append(cur)
                    cur, cur_n = [], 0
                groups[h].append([(t, GBUF_CHUNKS, True)])
                r -= GBUF_CHUNKS
            if cur_n + r > GBUF_CHUNKS and cur:
                groups[h].append(cur)
                cur, cur_n = [], 0
            cur.append((t, r, False))
            cur_n += r
        if cur:
            groups[h].append(cur)
    pl.groups = groups
    # schedule: all S groups first (build Ssb), then P groups containing tiles
    # >= PTILES (their finishes feed next layer's AG-S), then the rest.
    late = [gi for gi, g in enumerate(groups[0]) if any(t >= PTILES for (t, _, _) in g)]
    early = [gi for gi in range(len(groups[0])) if gi not in set(late)]
    sched = [(1, gi) for gi in range(len(groups[1]))]
    sched += [(0, gi) for gi in late + early]
    pl.sched = sched
    pl.n_sgroups = len(groups[1])
    pl.n_late = len(late)

    def wrap(flat):                                    # [num] -> [128, num//16]
        num = len(flat)
        w = np.zeros((16, num // 16), dtype=np.int16)
        w[np.arange(num) % 16, np.arange(num) // 16] = flat.astype(np.int16)
        return np.tile(w, (8, 1))

    # wrapped int16 index tensors [128, total_cols] per core
    idx_in = []
    col_ranges = [[], []]
    for c in range(NC):
        parts = []
        col = 0
        cursor = [0, 0]
        for h in range(2):
            for grp in groups[h]:
                nch = sum(r for (_, r, _) in grp)
                i0 = cursor[h] * P
                i1 = (cursor[h] + nch) * P
                cursor[h] += nch
                seg = slots[c][h][i0:i1]
                parts.append(wrap(seg))
                if c == 0:
                    col_ranges[h].append((col, col + len(seg) // 16))
                col += len(seg) // 16
        idx_in.append(np.concatenate(parts, axis=1))
    pl.idx_in = idx_in
    pl.col_ranges = col_ranges

    # permutation blocks: fold S-tiling partials into storage tiling.
    need = [set() for _ in range(TILES)]
    percore = [[dict() for _ in range(TILES)] for _ in range(NC)]
    for c in range(NC):
        for pidx in range(SHARD):
            n_ = node_at[c, pidx]
            if n_ < 0:
                continue
            q = posS[n_]
            t, l = pidx // P, pidx % P
            t1, l1 = q // P, q % P
            need[t].add(t1)
            percore[c][t].setdefault(t1, []).append((l1, l))
    block_t1 = [sorted(need[t]) for t in range(TILES)]
    NBLK = np.array([len(s) for s in block_t1], dtype=np.int64)
    TB = int(NBLK.sum())
    perm_in = []
    for c in range(NC):
        pm = np.zeros((P, TB * P), dtype=np.float32)
        bi = 0
        for t in range(TILES):
            for t1 in block_t1[t]:
                for (l1, l) in percore[c][t].get(t1, []):
                    pm[l1, bi * P + l] = 1.0
                bi += 1
        perm_in.append(pm.astype(bf16))
    pl.NBLK = NBLK
    pl.TB = TB
    pl.perm_in = perm_in
    pl.block_t1 = block_t1

    # per-tile lane mask: 1.0 for real nodes, 0.0 for dummies (used to zero
    # dummy lanes of the layer-2 table)
    mask = np.zeros((NC, P, TILES), dtype=np.float32)
    for c in range(NC):
        valid = (node_at[c] >= 0).astype(np.float32)
        mask[c] = valid.reshape(TILES, P).T
    pl.mask_in = mask
    return pl


# ----------------------------------------------------------------------------
# Bass kernel
# ----------------------------------------------------------------------------

def build_bass(pl, triv):
    import concourse.bacc as bacc
    import concourse.mybir as mybir
    import concourse.tile as tile
    from concourse.masks import make_identity

    f32 = mybir.dt.float32
    bf = mybir.dt.bfloat16
    AF = mybir.ActivationFunctionType
    OP = mybir.AluOpType

    nc = bacc.Bacc("TRN2", target_bir_lowering=False, debug=False, num_devices=NC)

    x_ext = nc.declare_dram_parameter("x", [SHARD, D], f32, isOutput=False)
    deg_ext = nc.declare_dram_parameter("deg", [P, TILES], f32, isOutput=False)
    mask_ext = nc.declare_dram_parameter("mask", [P, TILES], f32, isOutput=False)
    totcols = pl.idx_in[0].shape[1]
    idx_ext = nc.declare_dram_parameter("idx", [P, totcols], mybir.dt.int16, isOutput=False)
    perm_ext = nc.declare_dram_parameter("perm", [P, pl.TB * P], bf, isOutput=False)
    W_ext = [nc.declare_dram_parameter(f"W{i+1}", [D, D], f32, isOutput=False) for i in range(2)]
    vecs_ext = {}
    for nm in ["b1", "g1", "beta1", "b2", "g2", "beta2"]:
        vecs_ext[nm] = nc.declare_dram_parameter(nm, [P, D], f32, isOutput=False)
    out_ext = nc.declare_dram_parameter("out", [SHARD, D], f32, isOutput=True)

    NBMAX = int(pl.NBLK.max())

    with tile.TileContext(nc) as tc:
        with tc.tile_pool(name="const", bufs=1) as cpool, \
             tc.tile_pool(name="store", bufs=1) as spool, \
             tc.tile_pool(name="g", bufs=4) as gpool, \
             tc.tile_pool(name="work", bufs=3) as wpool, \
             tc.tile_pool(name="permp", bufs=4) as ppool, \
             tc.tile_pool(name="psA", bufs=4, space="PSUM") as psA, \
             tc.tile_pool(name="psB", bufs=2, space="PSUM") as psB, \
             tc.tile_pool(name="psC", bufs=2, space="PSUM") as psC, \
             tc.tile_pool(name="dram", bufs=1, space="DRAM") as dpool:

            ident32 = cpool.tile([P, P], f32)
            make_identity(nc, ident32[:])
            ident_bf = cpool.tile([P, P], bf)
            nc.vector.tensor_copy(out=ident_bf[:], in_=ident32[:])

            Wbf = []
            for i in range(2):
                wt = cpool.tile([P, D], f32, name=f"w32_{i}")
                nc.sync.dma_start(out=wt[:], in_=W_ext[i][:])
                wb = cpool.tile([P, D], bf, name=f"wbf_{i}")
                nc.vector.tensor_copy(out=wb[:], in_=wt[:])
                Wbf.append(wb)

            vecs = {}
            for nm in vecs_ext:
                vt = cpool.tile([P, D], f32, name=f"vec_{nm}")
                nc.sync.dma_start(out=vt[:], in_=vecs_ext[nm][:])
                vecs[nm] = vt

            deg_t = cpool.tile([P, TILES], f32)
            nc.sync.dma_start(out=deg_t[:], in_=deg_ext[:])
            sq = cpool.tile([P, TILES], f32)
            nc.scalar.activation(out=sq[:], in_=deg_t[:], func=AF.Sqrt)
            dinv = cpool.tile([P, TILES], f32)
            nc.vector.reciprocal(dinv[:], sq[:])
            mask_t = cpool.tile([P, TILES], f32)
            nc.sync.dma_start(out=mask_t[:], in_=mask_ext[:])
            dinvm = cpool.tile([P, TILES], f32)
            nc.vector.tensor_tensor(out=dinvm[:], in0=dinv[:], in1=mask_t[:],
                                    op=OP.mult)

            eps_t = cpool.tile([P, 1], f32)
            nc.vector.memset(eps_t[:], float(LN_EPS))
            idx_t = cpool.tile([P, totcols], mybir.dt.int16)
            nc.sync.dma_start(out=idx_t[:], in_=idx_ext[:])

            x_store = spool.tile([P, TILES, D], f32)
            xs_store = spool.tile([P, TILES, D], bf)
            for t0 in range(0, TILES, 7):
                t1b = min(t0 + 7, TILES)
                nc.sync.dma_start(
                    out=x_store[:, t0:t1b, :],
                    in_=x_ext[t0 * P:t1b * P, :].rearrange(
                        "(t l) f -> l t f", t=t1b - t0))
                for t in range(t0, t1b):
                    nc.scalar.activation(out=xs_store[:, t, :], in_=x_store[:, t, :],
                                         func=AF.Identity, scale=dinv[:, t:t + 1])

            h1g_store = spool.tile([P, TILES, D], bf)
            Ssb = spool.tile([P, TILES, D], bf)

            cc_inP = [dpool.tile([PREF, D], bf, name=f"ccinP{i}") for i in range(2)]
            cc_inS = [dpool.tile([SUF, D], bf, name=f"ccinS{i}") for i in range(2)]
            cc_outP = [dpool.tile([NC * PREF, D], bf, name=f"ccoutP{i}",
                                  addr_space="Shared") for i in range(2)]
            cc_outS = [dpool.tile([NC * SUF, D], bf, name=f"ccoutS{i}",
                                  addr_space="Shared") for i in range(2)]

            def emit_ag(layer, half):
                selfstore = xs_store if layer == 0 else h1g_store
                if half == 0:
                    trange, cin, cout = (0, PTILES), cc_inP[layer], cc_outP[layer]
                else:
                    trange, cin, cout = (PTILES, TILES), cc_inS[layer], cc_outS[layer]
                for t0 in range(trange[0], trange[1], 7):
                    t1b = min(t0 + 7, trange[1])
                    nc.sync.dma_start(
                        out=cin[(t0 - trange[0]) * P:(t1b - trange[0]) * P, :]
                            .rearrange("(t l) f -> l t f", t=t1b - t0),
                        in_=selfstore[:, t0:t1b, :])
                nc.gpsimd.collective_compute(
                    "AllGather", OP.bypass,
                    replica_groups=[list(range(NC))],
                    ins=[cin[:].opt()],
                    outs=[cout[:].opt()],
                )

            def finish_tile(layer, t, ps, started):
                selfstore = xs_store if layer == 0 else h1g_store
                nc.tensor.matmul(out=ps[:], lhsT=ident_bf[:],
                                 rhs=selfstore[:, t, :], start=not started, stop=False)
                bi0 = int(pl.NBLK[:t].sum())
                nb = len(pl.block_t1[t])
                assert nb >= 1
                pb = ppool.tile([P, NBMAX, P], bf, tag="pb", name=f"pb_{layer}_{t}")
                nc.sync.dma_start(out=pb[:, :nb, :],
                                  in_=perm_ext[:, bi0 * P:(bi0 + nb) * P])
                for j, t1 in enumerate(pl.block_t1[t]):
                    nc.tensor.matmul(out=ps[:], lhsT=pb[:, j, :], rhs=Ssb[:, t1, :],
                                     start=False, stop=(j == nb - 1))

                b_triv, g_triv, be_triv = triv[layer]
                s_agg = wpool.tile([P, D], bf, tag="sagg", name=f"sagg_{layer}_{t}")
                if b_triv:
                    nc.scalar.activation(out=s_agg[:], in_=ps[:], func=AF.Identity)
                else:
                    nc.scalar.activation(out=s_agg[:], in_=ps[:], func=AF.Identity,
                                         scale=dinv[:, t:t + 1])
                psT = psB.tile([P, D], bf, space="PSUM", tag="tr",
                               name=f"psT_{layer}_{t}")
                nc.tensor.transpose(out=psT[:], in_=s_agg[:], identity=ident_bf[:])
                s_aggT = wpool.tile([P, D], bf, tag="saggT", name=f"saggT_{layer}_{t}")
                nc.vector.tensor_copy(out=s_aggT[:], in_=psT[:])
                convp = psC.tile([P, D], f32, space="PSUM", tag="conv",
                                 name=f"conv_{layer}_{t}")
                nc.tensor.matmul(out=convp[:], lhsT=s_aggT[:], rhs=Wbf[layer][:],
                                 start=True, stop=True)

                if b_triv:
                    cb_ap = convp[:]
                else:
                    bv = vecs["b1" if layer == 0 else "b2"]
                    cb = wpool.tile([P, D], f32, tag="cb", name=f"cb_{layer}_{t}")
                    nc.vector.tensor_tensor(out=cb[:], in0=convp[:], in1=bv[:], op=OP.add)
                    cb_ap = cb[:]
                scr = wpool.tile([P, D], f32, tag="scr", name=f"scr_{layer}_{t}")
                negmu = wpool.tile([P, 1], f32, tag="negmu", name=f"negmu_{layer}_{t}")
                nc.scalar.activation(out=scr[:], in_=cb_ap, func=AF.Identity,
                                     scale=-1.0 / D, accum_out=negmu[:])
                ctr = wpool.tile([P, D], f32, tag="ctr", name=f"ctr_{layer}_{t}")
                nc.scalar.activation(out=ctr[:], in_=cb_ap, func=AF.Identity,
                                     bias=negmu[:, 0:1])
                sqs = wpool.tile([P, D], f32, tag="sqs", name=f"sqs_{layer}_{t}")
                var_raw = wpool.tile([P, 1], f32, tag="varr", name=f"varr_{layer}_{t}")
                nc.scalar.activation(out=sqs[:], in_=ctr[:], func=AF.Square,
                                     scale=float(1.0 / np.sqrt(D)),
                                     accum_out=var_raw[:])
                std = wpool.tile([P, 1], f32, tag="std", name=f"std_{layer}_{t}")
                nc.scalar.activation(out=std[:], in_=var_raw[:], func=AF.Sqrt,
                                     bias=eps_t[:, 0:1])
                rstd = wpool.tile([P, 1], f32, tag="rstd", name=f"rstd_{layer}_{t}")
                nc.vector.reciprocal(rstd[:], std[:])

                if not (g_triv and be_triv):
                    gv = vecs["g1" if layer == 0 else "g2"]
                    bev = vecs["beta1" if layer == 0 else "beta2"]
                    o1 = wpool.tile([P, D], f32, tag="o1", name=f"o1_{layer}_{t}")
                    nc.scalar.activation(out=o1[:], in_=ctr[:], func=AF.Identity,
                                         scale=rstd[:, 0:1])
                    o2 = wpool.tile([P, D], f32, tag="o2", name=f"o2_{layer}_{t}")
                    nc.vector.tensor_tensor(out=o2[:], in0=o1[:], in1=gv[:], op=OP.mult)
                    o3 = wpool.tile([P, D], f32, tag="o3", name=f"o3_{layer}_{t}")
                    nc.vector.tensor_tensor(out=o3[:], in0=o2[:], in1=bev[:], op=OP.add)
                    if layer == 0:
                        o4 = wpool.tile([P, D], f32, tag="o4", name=f"o4_{t}")
                        nc.scalar.activation(out=o4[:], in_=o3[:], func=AF.Relu)
                        nc.vector.tensor_scalar(out=h1g_store[:, t, :], in0=o4[:],
                                                scalar1=dinvm[:, t:t + 1], scalar2=None,
                                                op0=OP.mult)
                    else:
                        nc.sync.dma_start(out=out_ext[t * P:(t + 1) * P, :], in_=o3[:])
                else:
                    if layer == 0:
                        rsd = wpool.tile([P, 1], f32, tag="rsd", name=f"rsd_{t}")
                        nc.vector.tensor_scalar(out=rsd[:], in0=rstd[:],
                                                scalar1=dinvm[:, t:t + 1], scalar2=None,
                                                op0=OP.mult)
                        nc.scalar.activation(out=h1g_store[:, t, :], in_=ctr[:],
                                             func=AF.Relu, scale=rsd[:, 0:1])
                    else:
                        o1 = wpool.tile([P, D], f32, tag="o1", name=f"o1_{layer}_{t}")
                        nc.scalar.activation(out=o1[:], in_=ctr[:], func=AF.Identity,
                                             scale=rstd[:, 0:1])
                        nc.sync.dma_start(out=out_ext[t * P:(t + 1) * P, :], in_=o1[:])

            def run_layer(layer):
                tableP, tableS = cc_outP[layer], cc_outS[layer]
                nsched = len(pl.sched)
                # hook points inside layer 0 for layer 1's collectives:
                # AG-S(1) after the 'late' P-groups (tiles>=PTILES) + 1 margin;
                # AG-P(1) after the whole schedule.
                hook_s = pl.n_sgroups + pl.n_late + 1

                open_ps = {}
                for k in range(nsched):
                    if layer == 0 and k == hook_s:
                        emit_ag(1, 1)
                    h, gi = pl.sched[k]
                    grp = pl.groups[h][gi]
                    half_ap = tableP[:] if h == 0 else tableS[:]
                    c0, c1 = pl.col_ranges[h][gi]
                    nch = sum(r for (_, r, _) in grp)
                    gbuf = gpool.tile([P, GBUF_CHUNKS, D], bf, tag="g",
                                      name=f"g_{layer}_{h}_{gi}")
                    nc.gpsimd.dma_gather(
                        out_ap=gbuf[:, :nch, :],
                        in_ap=half_ap,
                        idxs_ap=idx_t[:, c0:c1],
                        num_idxs=nch * P,
                        num_idxs_reg=nch * P,
                        elem_size=D,
                        single_packet=False,
                    )
                    off = 0
                    for (t, r, partial) in grp:
                        key = (h, t)
                        if key in open_ps:
                            ps, started = open_ps.pop(key)
                        else:
                            ps = psA.tile([P, D], f32, space="PSUM", tag="agg",
                                          name=f"ps_{layer}_{h}_{t}")
                            started = False
                        for ri in range(r):
                            last = (not partial) and (h == 1) and (ri == r - 1)
                            nc.tensor.matmul(out=ps[:], lhsT=ident_bf[:],
                                             rhs=gbuf[:, off + ri, :],
                                             start=not started, stop=last)
                            started = True
                        off += r
                        if partial:
                            open_ps[key] = (ps, started)
                        elif h == 1:
                            nc.scalar.activation(out=Ssb[:, t, :], in_=ps[:],
                                                 func=AF.Identity)
                        else:
                            finish_tile(layer, t, ps, started)

                covered = set(t for grp in pl.groups[0] for (t, _, pa) in grp if not pa)
                for t in range(TILES):
                    if t not in covered:
                        ps = psA.tile([P, D], f32, space="PSUM", tag="agg",
                                      name=f"ps_{layer}_0z_{t}")
                        finish_tile(layer, t, ps, False)

            # layer-0 collectives up front (inputs ready after x load+scale);
            # Ssb zeroed once per layer (S groups cover all tiles with R>0).
            emit_ag(0, 1)
            emit_ag(0, 0)
            nc.vector.memset(Ssb[:], 0.0)
            run_layer(0)
            emit_ag(1, 0)
            nc.vector.memset(Ssb[:], 0.0)
            run_layer(1)

    nc.compile()
    return nc


# ----------------------------------------------------------------------------
# Entry point
# ----------------------------------------------------------------------------

_last_result = None


def kernel(**inputs) -> np.ndarray:
    edge_index = np.asarray(inputs["edge_index"])
    pl = build_plan(edge_index)

    from concourse.bass_utils import run_bass_kernel_spmd
    triv = []
    for i in (1, 2):
        triv.append((
            not np.any(np.asarray(inputs[f"b{i}"])),
            np.all(np.asarray(inputs[f"g{i}"]) == 1.0),
            not np.any(np.asarray(inputs[f"beta{i}"])),
        ))
    nc = build_bass(pl, triv)

    x = np.asarray(inputs["x"], dtype=np.float32)
    in_maps = []
    for c in range(NC):
        deg_t = np.ones((P, TILES), dtype=np.float32)
        xp = np.zeros((SHARD, D), dtype=np.float32)
        valid = pl.node_at[c] >= 0
        pidx = np.arange(SHARD)
        deg_t[pidx[valid] % P, pidx[valid] // P] = pl.deg[pl.node_at[c][valid]]
        xp[valid] = x[pl.node_at[c][valid]]
        m = {
            "x": xp,
            "deg": deg_t,
            "mask": pl.mask_in[c],
            "idx": pl.idx_in[c],
            "perm": pl.perm_in[c],
            "W1": np.asarray(inputs["W1"], np.float32),
            "W2": np.asarray(inputs["W2"], np.float32),
        }
        for nm in ["b1", "g1", "beta1", "b2", "g2", "beta2"]:
            m[nm] = np.tile(np.asarray(inputs[nm], np.float32)[None, :], (P, 1))
        in_maps.append(m)

    kw = {}
    if os.environ.get("KERNEL_TRACE") == "1":
        kw = dict(trace=True, trace_cores=[0])
    res = run_bass_kernel_spmd(nc, in_maps, core_ids=list(range(NC)), **kw)
    global _last_result
    _last_result = res

    out = np.zeros((N, D), dtype=np.float32)
    for c in range(NC):
        o = np.asarray(res.results[c]["out"], dtype=np.float32)
        valid = pl.node_at[c] >= 0
        out[pl.node_at[c][valid]] = o[valid]
    return out
